# revision 8
# baseline (speedup 1.0000x reference)
"""Causal self-attention (RMSNorm-QK + RoPE) Trainium2 Bass kernel.

Problem: B=2, T=2048, C=1024, H=16 heads, D=64.
Sharding: 8 cores = 2 (batch) x 4 (head groups of 4 heads).
Each core computes q/k/v projections for its 4 heads, attention, and a
partial output projection (column-parallel over heads); the host sums the
4 partials per batch and transposes.

All matmuls / elementwise run in float16 (tolerance is 2e-2; fp16 keeps the
softmax-exponent error ~1e-3).  PSUM accumulation is fp32.

Layout ("attention layout", channels on partitions, tokens free):
  q/k/v per chunk c in {0,1}: partitions = [head 2c d0..63, head 2c+1 d0..63].
  RoPE pairs (d, d+32) live at partitions p, p^32; the rotate-half operand
  comes from 4 SBUF->SBUF partition-swap DMAs.  K is NOT normalized; rms(k)
  is folded into the softmax exp as a per-partition (per-key-token)
  activation scale computed in transposed [token, head] form via N=2 matmuls.
  v 16 x [128, 260] f16: head h at cols 65h.., ones col at 65h+64 so the
  PV matmul accumulates the softmax denominator in Y row 64.

Pipeline: emission interleaves attention(j) heads with projection block
j+1 pieces (attention is Scalar-bound, projections Tensor-bound); the
S->exp->PV chain is software-pipelined (S_{k+1} issued before PV_k) so the
PE never waits on the exp; denominators of the 4 heads are batched into one
[4, T-block] Ln/Exp reciprocal on Scalar; out-projection runs one window
behind attention.
"""

import sys

for _p in ("/opt/trn_rl_repo",):
    if _p not in sys.path:
        sys.path.append(_p)

import numpy as np

B, T, C = 2, 2048, 1024
H_TOT, D = 16, 64
HPC = 4               # heads per core
N_CORES = 8
P = 128               # partitions
NB = 4                # t-blocks of 512
TB = 512              # t-block size
KCH = 8               # C / 128 contraction chunks
VW = 65 * HPC         # v width with ones columns = 260
RMS_EPS = 1.1920928955078125e-07
ROPE_BASE = 10000.0

_CACHE = {}


def _patch_act_tables():
    """Restrict ln/exp to the combined act-table set so bass's greedy
    first-match table pass emits a single LoadActFuncSet instead of
    ping-ponging (1.28us per reload).  Set indices are unchanged, only the
    (cached) set contents seen by the placement pass."""
    import concourse.bacc as bacc
    import concourse.hw_specs as hw_specs
    import concourse.mybir as mybir

    if getattr(bacc, "_act_tables_patched", False):
        return
    orig = hw_specs.get_activation_tables

    def patched(arch):
        tabs = dict(orig(arch))
        out = {}
        for name, s in tabs.items():
            s = set(s)
            if name != "natural_log_exp_and_others":
                s.discard(mybir.ActivationFunctionType.Ln)
                s.discard(mybir.ActivationFunctionType.Exp)
            out[name] = s
        return out

    bacc.get_activation_tables = patched
    bacc._act_tables_patched = True


def _build_consts():
    """Host-side constant tensors shared by all cores (fp16)."""
    inv_freq = (1.0 / (ROPE_BASE ** (np.arange(0, D, 2, dtype=np.float32) / np.float32(D)))).astype(np.float32)
    pos = np.arange(T, dtype=np.float32)
    freqs = np.outer(pos, inv_freq).astype(np.float32)      # [T, 32]
    cos = np.cos(freqs).astype(np.float32).T                # [32, T]
    sin = np.sin(freqs).astype(np.float32).T
    # cos2[p] = cos[p%32]; sin2[p] = +sin[p%32] for (p//32)%2==0 else -sin
    cos2 = np.tile(cos, (4, 1)).astype(np.float16)          # [128, T]
    sin2 = np.empty((P, T), dtype=np.float32)
    for g in range(4):
        sgn = 1.0 if g % 2 == 0 else -1.0
        sin2[32 * g:32 * g + 32] = sgn * sin
    sin2 = sin2.astype(np.float16)
    ind2 = np.zeros((P, 2), dtype=np.float16)               # col j: rows 64j..64j+63
    ind2[0:64, 0] = 1.0
    ind2[64:128, 1] = 1.0
    bc64 = np.zeros((2, P), dtype=np.float16)               # row j: cols 64j..
    bc64[0, 0:64] = 1.0
    bc64[1, 64:128] = 1.0
    sel4 = np.zeros((HPC, 256), dtype=np.float16)           # row h -> cols 64h..
    for h in range(HPC):
        sel4[h, 64 * h:64 * h + 64] = 1.0
    cossin = np.concatenate([cos2, sin2], axis=1)           # [128, 2T]
    return dict(cossin=cossin, ind2=ind2, bc64=bc64, sel4=sel4)


def _build_module():
    _patch_act_tables()
    import concourse.bacc as bacc
    import concourse.mybir as mybir
    import concourse.tile as tile

    f32 = mybir.dt.float32
    f16 = mybir.dt.float16
    Exp = mybir.ActivationFunctionType.Exp
    Ln = mybir.ActivationFunctionType.Ln
    Copy = mybir.ActivationFunctionType.Copy
    Alu = mybir.AluOpType

    nc = bacc.Bacc("TRN2", target_bir_lowering=False, debug=False,
                   num_devices=N_CORES)

    xt_d = nc.dram_tensor("xt", [C, T], f16, kind="ExternalInput").ap()
    wqk_d = nc.dram_tensor("wqk", [C, 512], f16, kind="ExternalInput").ap()
    wv_d = nc.dram_tensor("wv", [C, VW], f16, kind="ExternalInput").ap()
    wp_d = nc.dram_tensor("wp", [256, C], f16, kind="ExternalInput").ap()
    cossin_d = nc.dram_tensor("cossin", [P, 2 * T], f16, kind="ExternalInput").ap()
    ind2_d = nc.dram_tensor("ind2", [P, 2], f16, kind="ExternalInput").ap()
    bc64_d = nc.dram_tensor("bc64", [2, P], f16, kind="ExternalInput").ap()
    sel4_d = nc.dram_tensor("sel4", [HPC, 256], f16, kind="ExternalInput").ap()
    out_d = nc.dram_tensor("outT", [C, T], f16, kind="ExternalOutput").ap()

    with tile.TileContext(nc) as tc:
        with (
            tc.tile_pool(name="sb", bufs=1) as sb,
            tc.tile_pool(name="tr", bufs=2) as tr,
            tc.tile_pool(name="ps", bufs=2, space="PSUM") as ps,
        ):
            # ---------------- persistent tiles + loads ----------------
            def load(name, dram_slice, shape, dt=f16):
                t = sb.tile(shape, dt, tag=name, name=name)
                nc.sync.dma_start(out=t[:], in_=dram_slice)
                return t

            ind2_t = load("ind2", ind2_d[:, :], [P, 2])
            bc64_t = load("bc64", bc64_d[:, :], [2, P])
            sel4_t = load("sel4", sel4_d[:, :], [HPC, 256])
            # wq|wk packed in one dram tensor: halves the startup DMA count
            wqk_t = [load(f"wqk{k}", wqk_d[k * P:(k + 1) * P, :], [P, 512])
                     for k in range(KCH)]
            wq_t = [t[:, 0:256] for t in wqk_t]
            wk_t = [t[:, 256:512] for t in wqk_t]
            # x in [128, 1024] half-tiles: halves 0 (blocks 0-1) on the Sync
            # DGE up front, halves 1 (blocks 2-3) on the Scalar DGE.
            x_t = [[sb.tile([P, 2 * TB], f16, tag=f"x{k}_{hf}", name=f"x{k}_{hf}")
                    for hf in range(2)] for k in range(KCH)]
            for k in range(KCH):
                nc.sync.dma_start(out=x_t[k][0][:],
                                  in_=xt_d[k * P:(k + 1) * P, 0:2 * TB])
            cossin_t = load("cossin", cossin_d[:, :], [P, 2 * T])
            wv_t = [load(f"wv{k}", wv_d[k * P:(k + 1) * P, :], [P, VW])
                    for k in range(KCH)]
            wp_t = [load(f"wp{c}", wp_d[c * P:(c + 1) * P, :], [P, C])
                    for c in range(2)]
            for k in range(KCH):
                nc.scalar.dma_start(out=x_t[k][1][:],
                                    in_=xt_d[k * P:(k + 1) * P, 2 * TB:4 * TB])

            epsq = sb.tile([2, 1], f32, tag="epsq", name="epsq")
            nc.gpsimd.memset(epsq[:], RMS_EPS)
            epsk = sb.tile([P, 1], f32, tag="epsk", name="epsk")
            nc.gpsimd.memset(epsk[:], 64.0 * RMS_EPS)

            # persistent intermediates
            rcq_t = [sb.tile([P, T], f16, tag=f"rcq{c}", name=f"rcq{c}")
                     for c in range(2)]
            rck_t = [sb.tile([P, T], f16, tag=f"rck{c}", name=f"rck{c}")
                     for c in range(2)]
            yT_t = [sb.tile([P, T], f16, tag=f"yT{c}", name=f"yT{c}")
                    for c in range(2)]
            v_t = [sb.tile([P, VW], f16, tag=f"v{s}", name=f"v{s}")
                   for s in range(T // P)]
            invkT_t = [[sb.tile([P, 8], f32, tag=f"ikT{n}_{c}", name=f"ikT{n}_{c}")
                        for c in range(2)] for n in range(NB)]

            # ---------------- phase-1 pieces ----------------
            def p1_qk(n, which):
                """Projection + stats + rope for q (which='q') or k ('k')."""
                nsl = slice(n * TB, (n + 1) * TB)
                hf, tl = n // 2, (n % 2) * TB
                w_t = wq_t if which == "q" else wk_t
                pp = [ps.tile([P, TB], f32, tag="p1", name=f"p{which}{n}_{c}",
                              bufs=2) for c in range(2)]
                for c in range(2):
                    for k in range(KCH):
                        nc.tensor.matmul(pp[c][:],
                                         lhsT=w_t[k][:, c * P:(c + 1) * P],
                                         rhs=x_t[k][hf][:, tl:tl + TB], start=(k == 0),
                                         stop=(k == KCH - 1))
                xm = tr.tile([P, 2 * TB], f16, tag="xm", name=f"x{which}{n}",
                             bufs=2)
                for c in range(2):
                    nc.vector.tensor_copy(xm[:, c * TB:(c + 1) * TB], pp[c][:])
                sq = tr.tile([P, 2 * TB], f16, tag="sqm", name=f"sq{which}{n}",
                             bufs=2)
                nc.vector.tensor_mul(sq[:], xm[:], xm[:])

                if which == "q":
                    src = tr.tile([P, 2 * TB], f16, tag="xnm", name=f"xn{n}",
                                  bufs=2)
                    for c in range(2):
                        csl = slice(c * TB, (c + 1) * TB)
                        msum = ps.tile([2, TB], f32, tag="pst",
                                       name=f"ms{n}_{c}", bufs=2)
                        nc.tensor.matmul(msum[:], lhsT=ind2_t[:],
                                         rhs=sq[:, csl], start=True, stop=True)
                        invr = tr.tile([2, TB], f16, tag="invr",
                                       name=f"ivr{n}_{c}", bufs=2)
                        nc.scalar.activation(invr[:], msum[:], Ln,
                                             bias=epsq[:], scale=1.0 / 64.0)
                        nc.scalar.activation(invr[:], invr[:], Exp, scale=-0.5)
                        inv128 = ps.tile([P, TB], f32, tag="p1",
                                         name=f"iv{n}_{c}", bufs=2)
                        nc.tensor.matmul(inv128[:], lhsT=bc64_t[:],
                                         rhs=invr[:], start=True, stop=True)
                        nc.vector.tensor_mul(src[:, csl], xm[:, csl],
                                             inv128[:])
                else:
                    src = xm
                    for c in range(2):
                        kst = ps.tile([P, 8], f32, tag="pst",
                                      name=f"kst{n}_{c}", bufs=2)
                        for sc in range(4):
                            nc.tensor.matmul(
                                kst[:, 2 * sc:2 * sc + 2],
                                lhsT=sq[:, c * TB + sc * P:c * TB + (sc + 1) * P],
                                rhs=ind2_t[:], start=True, stop=True)
                        nc.scalar.activation(invkT_t[n][c][:], kst[:], Ln,
                                             bias=epsk[:])
                        nc.scalar.activation(invkT_t[n][c][:],
                                             invkT_t[n][c][:], Exp, scale=-0.5)

                xsw = tr.tile([P, 2 * TB], f16, tag="xsw",
                              name=f"x{which}sw{n}", bufs=2)
                for g in range(2):
                    a, b = 64 * g, 64 * g + 32
                    nc.sync.dma_start(out=xsw[a:a + 32, :], in_=src[b:b + 32, :])
                    nc.sync.dma_start(out=xsw[b:b + 32, :], in_=src[a:a + 32, :])
                rc_t = rcq_t if which == "q" else rck_t
                for c in range(2):
                    csl = slice(c * TB, (c + 1) * TB)
                    t1 = tr.tile([P, TB], f16, tag="t12",
                                 name=f"t1{which}{n}_{c}", bufs=3)
                    t2 = tr.tile([P, TB], f16, tag="t12",
                                 name=f"t2{which}{n}_{c}", bufs=3)
                    nc.vector.tensor_mul(t1[:], src[:, csl], cossin_t[:, nsl])
                    nc.vector.tensor_mul(t2[:], xsw[:, csl], cossin_t[:, T + n * TB:T + (n + 1) * TB])
                    nc.vector.tensor_add(rc_t[c][:, nsl], t1[:], t2[:])

            def p1_v(n):
                hf, tl = n // 2, (n % 2) * TB
                for s_rel in range(4):
                    pv = ps.tile([P, VW], f32, tag="p1", name=f"pv{n}_{s_rel}",
                                 bufs=2)
                    for k in range(KCH):
                        nc.tensor.matmul(
                            pv[:],
                            lhsT=x_t[k][hf][:, tl + s_rel * P:tl + (s_rel + 1) * P],
                            rhs=wv_t[k][:], start=(k == 0), stop=(k == KCH - 1))
                    vt = v_t[4 * n + s_rel]
                    nc.vector.tensor_copy(vt[:], pv[:])
                    nc.vector.tensor_scalar(vt[:, 64:VW:65], pv[:, 64:VW:65],
                                            0.0, 1.0, Alu.mult, Alu.add)

            # ---------------- attention ----------------
            def attn_head(h, j, den4):
                """S->exp->PV software-pipelined; den row lands in den4[h]."""
                cch, half = h // 2, h % 2
                rsl = slice(64 * half, 64 * half + 64)
                n_k = 4 * (j + 1)
                Y = ps.tile([65, TB], f32, tag="py", name=f"Y{h}_{j}", bufs=2)
                pend = None  # (e0, mt, k)
                for k in range(n_k):
                    r = k - 4 * j
                    mt = 128 * r if r > 0 else 0
                    S = ps.tile([P, TB], f32, tag="ps", name=f"S{h}_{j}_{k}",
                                bufs=2)
                    nc.tensor.matmul(
                        S[:, mt:], lhsT=rck_t[cch][rsl, k * P:(k + 1) * P],
                        rhs=rcq_t[cch][rsl, j * TB + mt:(j + 1) * TB],
                        start=True, stop=True)
                    e0 = tr.tile([P, TB], f16, tag="e0", name=f"e{h}_{j}_{k}",
                                 bufs=3)
                    nc.scalar.activation(
                        e0[:, mt:], S[:, mt:], Exp,
                        scale=invkT_t[k // 4][cch][:, 2 * (k % 4) + half:
                                                   2 * (k % 4) + half + 1])
                    if r >= 0:
                        nc.gpsimd.affine_select(
                            out=e0[:, 128 * r:128 * r + 128],
                            in_=e0[:, 128 * r:128 * r + 128],
                            pattern=[[1, 128]], compare_op=Alu.is_ge,
                            fill=0.0, base=0, channel_multiplier=-1)
                    if pend is not None:
                        pe0, pmt, pk = pend
                        nc.tensor.matmul(Y[:, pmt:],
                                         lhsT=v_t[pk][:, 65 * h:65 * h + 65],
                                         rhs=pe0[:, pmt:], start=(pk == 0),
                                         stop=False)
                    pend = (e0, mt, k)
                pe0, pmt, pk = pend
                nc.tensor.matmul(Y[:, pmt:], lhsT=v_t[pk][:, 65 * h:65 * h + 65],
                                 rhs=pe0[:, pmt:], start=(pk == 0), stop=True)
                # y rows to sbuf (f32: pre-normalization values can be large),
                # denominator row into the window-shared den4 tile.
                yraw = tr.tile([65, TB], f32, tag="yrw", name=f"yr{h}_{j}",
                               bufs=5, padded_shape=[P, TB])
                nc.vector.tensor_copy(yraw[:], Y[:])
                nc.sync.dma_start(out=den4[h:h + 1, :], in_=yraw[64:65, :])
                return yraw

            def attn_tail(j, den4, yraws):
                """Batched denominator reciprocal + normalize + yT scatter."""
                jsl = slice(j * TB, (j + 1) * TB)
                invd = tr.tile([HPC, TB], f16, tag="invd", name=f"invd{j}",
                               bufs=2)
                nc.scalar.activation(invd[:], den4[:], Ln)
                nc.scalar.activation(invd[:], invd[:], Exp, scale=-1.0)
                for h in range(HPC):
                    cch, half = h // 2, h % 2
                    rsl = slice(64 * half, 64 * half + 64)
                    bcD = ps.tile([64, TB], f32, tag="ps", name=f"bcD{h}_{j}",
                                  bufs=2)
                    nc.tensor.matmul(bcD[:], lhsT=sel4_t[:, 64 * h:64 * h + 64],
                                     rhs=invd[:], start=True, stop=True)
                    yn = tr.tile([64, TB], f16, tag="ynm", name=f"yn{h}_{j}",
                                 bufs=2, padded_shape=[P, TB])
                    nc.vector.tensor_mul(yn[:], yraws[h][0:64, :], bcD[:])
                    nc.sync.dma_start(out=yT_t[cch][rsl, jsl], in_=yn[:])

            # ---------------- out-projection ----------------
            def p4(j):
                jsl = slice(j * TB, (j + 1) * TB)
                for o in range(8):
                    osl = slice(o * P, (o + 1) * P)
                    po = ps.tile([P, TB], f32, tag="p1", name=f"po{j}_{o}",
                                 bufs=2)
                    nc.tensor.matmul(po[:], lhsT=wp_t[0][:, osl],
                                     rhs=yT_t[0][:, jsl], start=True, stop=False)
                    nc.tensor.matmul(po[:], lhsT=wp_t[1][:, osl],
                                     rhs=yT_t[1][:, jsl], start=False, stop=True)
                    ob = tr.tile([P, TB], f16, tag="ob", name=f"ob{j}_{o}",
                                 bufs=3)
                    nc.vector.tensor_copy(ob[:], po[:])
                    nc.sync.dma_start(out=out_d[osl, jsl], in_=ob[:])

            # ---------------- schedule ----------------
            # window 0: projections for block 0 only
            p1_qk(0, "q")
            p1_qk(0, "k")
            p1_v(0)
            # windows 1..NB: attention j = w-1 interleaved with p1(w) pieces
            for w in range(1, NB + 1):
                j = w - 1
                den4 = tr.tile([HPC, TB], f32, tag="den4", name=f"den4_{j}",
                               bufs=2)
                yraws = []
                yraws.append(attn_head(0, j, den4))
                if w < NB:
                    p1_qk(w, "q")
                yraws.append(attn_head(1, j, den4))
                if w < NB:
                    p1_qk(w, "k")
                yraws.append(attn_head(2, j, den4))
                if w < NB:
                    p1_v(w)
                yraws.append(attn_head(3, j, den4))
                if w >= 2:
                    p4(w - 2)        # out-projection one window behind
                attn_tail(j, den4, yraws)
            p4(NB - 1)

    nc.compile()
    return nc


def _get_module():
    if "nc" not in _CACHE:
        _CACHE["nc"] = _build_module()
        _CACHE["consts"] = _build_consts()
    return _CACHE["nc"], _CACHE["consts"]


def _core_inputs(x, w_q, w_k, w_v, w_proj, core):
    """Build the per-core input map (numpy fp16, host-side sharding)."""
    b = core // 4
    g = core % 4
    heads = [4 * g + j for j in range(HPC)]

    xt = np.ascontiguousarray(x[b].T).astype(np.float16)     # [C, T]

    # attention-layout column perm: col m of chunk c -> head 2c+(m//64), dim m%64
    perm = np.empty(256, dtype=np.int64)
    for m in range(256):
        c, mm = m // 128, m % 128
        perm[m] = 64 * heads[2 * c + mm // 64] + (mm % 64)
    wqk = np.concatenate([w_q[perm, :].T, w_k[perm, :].T],
                         axis=1).astype(np.float16)              # [C, 512]

    wv_aug = np.zeros((C, VW), dtype=np.float32)
    for j in range(HPC):
        wv_aug[:, 65 * j:65 * j + 64] = w_v[64 * heads[j]:64 * heads[j] + 64, :].T
    wv = wv_aug.astype(np.float16)

    wp = np.ascontiguousarray(w_proj[:, perm].T).astype(np.float16)  # [256, C]

    return dict(xt=xt, wqk=wqk, wv=wv, wp=wp)


def kernel(x, w_q, w_k, w_v, w_proj, _trace=False, _trace_cores=None):
    from concourse.bass_utils import run_bass_kernel_spmd

    nc, consts = _get_module()
    x = np.asarray(x, dtype=np.float32)
    in_maps = []
    for core in range(N_CORES):
        m = _core_inputs(np.asarray(x), np.asarray(w_q), np.asarray(w_k),
                         np.asarray(w_v), np.asarray(w_proj), core)
        m.update(consts)
        in_maps.append(m)

    res = run_bass_kernel_spmd(nc, in_maps, list(range(N_CORES)),
                               trace=_trace, trace_cores=_trace_cores)
    outs = [res.results[c]["outT"] for c in range(N_CORES)]
    out = np.empty((B, T, C), dtype=np.float32)
    for b in range(B):
        acc = outs[4 * b].astype(np.float32)
        for g in range(1, 4):
            acc = acc + outs[4 * b + g].astype(np.float32)
        out[b] = acc.T
    if _trace:
        kernel._last_exec_time_ns = res.exec_time_ns
        kernel._last_results = res
    return out
